# revision 31
# baseline (speedup 1.0000x reference)
"""Grouped-Query Attention (Gemma3-style, sliding-window) Trainium2 kernel.

Sharding: 8 cores = (batch b in {0,1}) x (kv-group G in {0..3}).
Each core computes, for its batch's tokens:
  - k/v projections for group G, q projections for heads {G, G+4},
  - qk-rmsnorm, sliding-window causal attention for its 2 heads,
  - partial output projection through the matching 512 rows of Wo.
Host sums the 4 partials per batch.

The q/k/v projections and the output projection run as fp8(e4m3)
DoubleRow matmuls (cost model: 0.5 cycles per output element, K=256 per
instruction) with full hi/lo residual correction:
  x = xh + xlo/16,  W = Whi/s + Wlo/s   (per-path scale s)
  x@W = xh@Whi + [xlo@(Whi/16) + xh@Wlo]   (lo*lo dropped)

Scores run fp8-DR with k-side residual correction only:
  q8 = q8(4*qhat), k8 = q8(4*khat), klo = q8(4*khat - k8)
  S = k8.T@q8 + klo.T@q8 = 16*qhat.khat + eps,  exp(S/256 - 2)
(quantized AFTER the PE transposes so no extra transpose work; one DR
per term per j-tile with both heads packed in N=256).  Numerics (CPU
emulation, matches HW to ~1%): this lands at max-rel-err 1.33e-2 vs
the 2e-2 gate; dropping any other correction pass (proj x/W, out-proj
a/W, fp8 PV) individually pushes past ~2.4e-2, so those stay.
Softmax/PV stay fp16.

Schedule/structure:
  - per-projection psum chains (k, v, q in separate 2-buf pools) +
    split rmsnorm so the startup only waits for the weights each chain
    actually needs; single SP DMA queue with need-ordered weight
    stream; tile-2's prologue chains borrow the idle out-proj psum
  - sliding-window masks applied as Pool affine_select zeroing on the
    exp output instead of PE mask-add matmuls
  - out-projection emitted as a job queue of [128,512] regions popped
    at the next iteration's known PE stall points (after norms/exp),
    so PSUM-drain and DVE-quant latency hide behind other PE work
  - pv(h1) runs before drain(h0) so the drain's DVE ao-scale latency
    hides under PE work; a2 quantization rides the idle Pool engine
  - output DMA: two [128,1536/1024] transfers per tile on the SP
    hardware DGE queue (per-region for the last tile to cut the tail)

Scales: proj psums hold 64*q, 64*k, 256*v (rmsnorm is scale-invariant,
V copy divides by 256). Out-proj psum holds 16*out; drain divides
by 16.
"""

import math
from collections import deque
from contextlib import ExitStack

import numpy as np

import concourse.bass as bass
import concourse.tile as tile
from concourse import bacc, mybir
from concourse.bass import ts, ds
from concourse.bass_utils import run_bass_kernel_spmd
from concourse.masks import make_identity

F8 = mybir.dt.float8e4
F16 = mybir.dt.float16
F32 = mybir.dt.float32
AF = mybir.ActivationFunctionType
ALU = mybir.AluOpType
DR = mybir.MatmulPerfMode.DoubleRow
_MY_FUNCS = {AF.Exp, AF.Ln, AF.Copy, AF.Square}

# Steer Bacc's activation-table chooser so Square/Ln/Exp/Copy all resolve to
# the one function set that contains them all (natural_log_exp_and_others).
import concourse.bacc as _bacc_mod
from concourse.hw_specs import get_activation_tables as _orig_gat

_ONE_SET = "natural_log_exp_and_others"


def _steered_gat(arch):
    tabs = _orig_gat(arch)
    if _ONE_SET not in tabs:
        return tabs
    return {name: (set(funcs) if name == _ONE_SET else set(funcs) - _MY_FUNCS)
            for name, funcs in tabs.items()}


_bacc_mod.get_activation_tables = _steered_gat

EPS = 1e-6
HD = 256  # head dim
EXP_BIAS = -2.0


def build_nc(T=2048, D=2560, WIN=1024, fast_scale=True):
    nT, nD, WT = T // 128, D // 128, WIN // 128
    nP = nD // 2  # chunk pairs
    nc = bacc.Bacc("TRN2", target_bir_lowering=False, debug=False)

    xcomb = nc.dram_tensor("xcomb", [nT, 128, 2, nP, 2, 128], F8,
                           kind="ExternalInput").ap()
    wq8 = nc.dram_tensor("wq8", [128, nP, 2, 512], F8, kind="ExternalInput").ap()
    wqc = nc.dram_tensor("wqc", [128, nP, 2, 2, 512], F8, kind="ExternalInput").ap()
    wk8 = nc.dram_tensor("wk8", [128, nP, 2, 256], F8, kind="ExternalInput").ap()
    wkc = nc.dram_tensor("wkc", [128, nP, 2, 2, 256], F8, kind="ExternalInput").ap()
    wv8 = nc.dram_tensor("wv8", [128, nP, 2, 256], F8, kind="ExternalInput").ap()
    wvc = nc.dram_tensor("wvc", [128, nP, 2, 2, 256], F8, kind="ExternalInput").ap()
    wo4 = nc.dram_tensor("wo4", [128, 2, 2, D], F8, kind="ExternalInput").ap()
    w2 = nc.dram_tensor("w2", [128, 2, 2, D], F8, kind="ExternalInput").ap()
    qs = nc.dram_tensor("qs", [128, 2], F32, kind="ExternalInput").ap()
    ks = nc.dram_tensor("ks", [128, 2], F32, kind="ExternalInput").ap()
    outp = nc.dram_tensor("outp", [T, D], F16, kind="ExternalOutput").ap()

    with tile.TileContext(nc) as tc, ExitStack() as ctx:
        _body(ctx, tc, nT, nP, WT, D, fast_scale,
              xcomb, wq8, wqc, wk8, wkc, wv8, wvc, wo4, w2, qs, ks, outp)

    nc.compile()
    return nc


def _body(ctx, tc, nT, nP, WT, D, fast_scale,
          xcomb, wq8, wqc, wk8, wkc, wv8, wvc, wo4, w2, qs, ks, outp):
    nc = tc.nc

    const = ctx.enter_context(tc.tile_pool(name="const", bufs=1))
    acts = ctx.enter_context(tc.tile_pool(name="acts", bufs=1))
    work = ctx.enter_context(tc.tile_pool(name="work", bufs=4))
    nrm = ctx.enter_context(tc.tile_pool(name="nrm", bufs=3))
    ptp_pool = ctx.enter_context(tc.tile_pool(name="ptp", bufs=8))
    aop = ctx.enter_context(tc.tile_pool(name="aop", bufs=4))
    stats = ctx.enter_context(tc.tile_pool(name="stats", bufs=8))
    # 8 PSUM banks: kv 2 + q 2 + att (stg/po/ptao/ptkq rotation) 2 + out 2.
    # Tile-2's prologue chains borrow the (still unused) out slots so the
    # startup can run 3 tiles of projection lookahead with 2-buf pools.
    ps_kv = ctx.enter_context(tc.tile_pool(name="ps_kv", bufs=2,
                                           space="PSUM"))
    ps_q = ctx.enter_context(tc.tile_pool(name="ps_q", bufs=2, space="PSUM"))
    ps_att = ctx.enter_context(tc.tile_pool(name="ps_att", bufs=2,
                                            space="PSUM"))
    ps_out = ctx.enter_context(tc.tile_pool(name="ps_out", bufs=2,
                                            space="PSUM"))

    ident = const.tile([128, 128], F16, tag="ident")
    make_identity(nc, ident[:])
    bias_eps = const.tile([128, 1], F32, tag="bias_eps")
    nc.vector.memset(bias_eps[:], EPS)
    bias_m2 = const.tile([128, 1], F32, tag="bias_m2")
    nc.vector.memset(bias_m2[:], EXP_BIAS)
    qs_sb = const.tile([128, 2], F32, tag="qs")
    ks_sb = const.tile([128, 2], F32, tag="ks")
    qs4_sb = const.tile([128, 2], F32, tag="qs4")
    ks4_sb = const.tile([128, 2], F32, tag="ks4")

    # one tile per weight tensor: few big DMAs beat many chunked ones
    wq8_sb = const.tile([128, nP, 2, 512], F8, tag="wq8")
    wk8_sb = const.tile([128, nP, 2, 256], F8, tag="wk8")
    wv8_sb = const.tile([128, nP, 2, 256], F8, tag="wv8")
    wqc_sb = const.tile([128, nP, 2, 2, 512], F8, tag="wqc")
    wkc_sb = const.tile([128, nP, 2, 2, 256], F8, tag="wkc")
    wvc_sb = const.tile([128, nP, 2, 2, 256], F8, tag="wvc")
    wq8_c = [wq8_sb[:, c] for c in range(nP)]
    wk8_c = [wk8_sb[:, c] for c in range(nP)]
    wv8_c = [wv8_sb[:, c] for c in range(nP)]
    wqc_c = [wqc_sb[:, c] for c in range(nP)]
    wkc_c = [wkc_sb[:, c] for c in range(nP)]
    wvc_c = [wvc_sb[:, c] for c in range(nP)]
    wo4_sb = const.tile([128, 2, 2, D], F8, tag="wo4")
    w2_sb = const.tile([128, 2, 2, D], F8, tag="w2")

    # full-length fp8 q/k for the scores (hi at scale 4, k residual lo);
    # numerics: scores = q8@k8 + q8@klo, exp scale 1/256 (rel-err 1.35e-2)
    # QT layout: [d, c(2), i, h(2), t]  (c = head-dim chunk)
    QT = acts.tile([128, 2, nT, 2, 128], F8, tag="QT")
    KT = acts.tile([128, 2, nT * 128], F8, tag="KT")   # [d, c, t] hi
    KL = acts.tile([128, 2, nT * 128], F8, tag="KL")   # [d, c, t] residual
    V = [acts.tile([128, 257], F16, tag=f"v{j}", name=f"v{j}")
         for j in range(nT)]  # last column is ones (softmax row sums)
    for j in range(nT):
        nc.gpsimd.memset(V[j][:, 256:257], 1.0)

    state = {}
    xt_tiles = {}
    jobs = deque()

    def xt_dma_emit(i):
        xt_i = work.tile([128, 2, nP, 2, 128], F8, tag="xt", name="xt_i")
        # hi half first: main-pass matmuls only need xh
        nc.sync.dma_start(xt_i[:, 1], xcomb[i][:, 1])
        nc.sync.dma_start(xt_i[:, 0], xcomb[i][:, 0])
        xt_tiles[i] = xt_i

    def kv_k_mains(i, pool=None, tag="pkv"):
        xt_i = xt_tiles[i]
        ps_kvt = (pool or ps_kv).tile([128, 512], F32, tag=tag, name="ps_kvt")
        sl = ps_kvt[:, 0:256]
        for cp in range(nP):
            nc.tensor.matmul(sl, lhsT=xt_i[:, 1, cp, :, :],
                             rhs=wk8_c[cp], start=(cp == 0), stop=False,
                             perf_mode=DR)
        state[("pskv", i)] = ps_kvt

    def kv_k_corrs(i):
        xt_i = xt_tiles[i]
        sl = state[("pskv", i)][:, 0:256]
        for c in range(2 * nP):
            cp, p = c // 2, c % 2
            nc.tensor.matmul(sl, lhsT=xt_i[:, :, cp, p, :],
                             rhs=wkc_c[cp][:, :, p, :], start=False,
                             stop=(c == 2 * nP - 1), perf_mode=DR)

    def kv_k_chain(i, pool=None, tag="pkv"):
        kv_k_mains(i, pool, tag)
        kv_k_corrs(i)

    def kv_v_mains(i):
        xt_i = xt_tiles[i]
        sl = state[("pskv", i)][:, 256:512]
        for cp in range(nP):
            nc.tensor.matmul(sl, lhsT=xt_i[:, 1, cp, :, :],
                             rhs=wv8_c[cp], start=(cp == 0), stop=False,
                             perf_mode=DR)

    def kv_v_corrs(i):
        xt_i = xt_tiles[i]
        sl = state[("pskv", i)][:, 256:512]
        for c in range(2 * nP):
            cp, p = c // 2, c % 2
            nc.tensor.matmul(sl, lhsT=xt_i[:, :, cp, p, :],
                             rhs=wvc_c[cp][:, :, p, :], start=False,
                             stop=(c == 2 * nP - 1), perf_mode=DR)

    def kv_v_chain(i):
        kv_v_mains(i)
        kv_v_corrs(i)

    def q_mains(i, pool=None, tag="pq2"):
        xt_i = xt_tiles[i]
        ps_qt = (pool or ps_q).tile([128, 512], F32, tag=tag, name="ps_qt")
        for cp in range(nP):
            nc.tensor.matmul(ps_qt[:], lhsT=xt_i[:, 1, cp, :, :],
                             rhs=wq8_c[cp], start=(cp == 0), stop=False,
                             perf_mode=DR)
        state[("psq", i)] = ps_qt

    def q_corrs(i):
        xt_i = xt_tiles.pop(i)
        ps_qt = state[("psq", i)]
        for c in range(2 * nP):
            cp, p = c // 2, c % 2
            nc.tensor.matmul(ps_qt[:], lhsT=xt_i[:, :, cp, p, :],
                             rhs=wqc_c[cp][:, :, p, :], start=False,
                             stop=(c == 2 * nP - 1), perf_mode=DR)

    def norm_kv(i):
        ps_kvt = state.pop(("pskv", i))
        # V tile: psum holds 256*v
        nc.vector.tensor_scalar_mul(V[i][:, 0:256], ps_kvt[:, 256:512],
                                    1.0 / 256.0)
        # k rmsnorm (scale-invariant): rinv = exp(-0.5*ln(ssq/256 + eps))
        sstk = stats.tile([128, 1], F32, tag="sstk", name="sstk")
        sq = nrm.tile([128, 256], F32, tag="sq", name="sq")
        nc.scalar.activation(sq[:], ps_kvt[:, 0:256], AF.Square,
                             accum_out=sstk[:])
        lnk = stats.tile([128, 1], F32, tag="lnk", name="lnk")
        nc.scalar.activation(lnk[:], sstk[:], AF.Ln, bias=bias_eps[:],
                             scale=1.0 / 256.0)
        rk = stats.tile([128, 1], F32, tag="rk", name="rk")
        nc.scalar.activation(rk[:], lnk[:], AF.Exp, scale=-0.5)
        kn = nrm.tile([128, 256], F16, tag="kn", name="kn")
        nc.vector.tensor_scalar_mul(kn[:], ps_kvt[:, 0:256], rk[:])
        state[("kn", i)] = kn

    def transp_k(i):
        kn = state.pop(("kn", i))
        ptkq = ps_att.tile([128, 6, 128], F16, tag="sa", name="ptkq")
        for cc in range(2):
            nc.tensor.transpose(ptkq[:, 4 + cc, :], kn[:, ts(cc, 128)],
                                ident[:])
        # quantize after transpose: k8 = q8(4*k), klo = q8(4*k - k8)
        if fast_scale:
            nc.vector.tensor_scalar_mul(KT[:, :, ts(i, 128)],
                                        ptkq[:, 4:6, :], 4.0)
            nc.vector.scalar_tensor_tensor(KL[:, :, ts(i, 128)],
                                           ptkq[:, 4:6, :], 4.0,
                                           KT[:, :, ts(i, 128)],
                                           ALU.mult, ALU.subtract)
        else:
            for cc in range(2):
                nc.vector.tensor_scalar_mul(KT[:, cc, ts(i, 128)],
                                            ptkq[:, 4 + cc, :],
                                            ks4_sb[:, ds(cc, 1)])
                nc.vector.scalar_tensor_tensor(KL[:, cc, ts(i, 128)],
                                               ptkq[:, 4 + cc, :],
                                               ks4_sb[:, ds(cc, 1)],
                                               KT[:, cc, ts(i, 128)],
                                               ALU.mult, ALU.subtract)
        state[("ptkq", i)] = ptkq

    def norm_q(i):
        ps_qt = state.pop(("psq", i))
        sstq = stats.tile([128, 2], F32, tag="sstq", name="sstq")
        for jj in range(2):
            sq = nrm.tile([128, 256], F32, tag="sq", name="sq")
            nc.scalar.activation(sq[:], ps_qt[:, ts(jj, 256)], AF.Square,
                                 accum_out=sstq[:, jj:jj + 1])
        lnq = stats.tile([128, 2], F32, tag="lnq", name="lnq")
        nc.scalar.activation(lnq[:], sstq[:], AF.Ln, bias=bias_eps[:],
                             scale=1.0 / 256.0)
        rq = stats.tile([128, 2], F32, tag="rq", name="rq")
        nc.scalar.activation(rq[:], lnq[:], AF.Exp, scale=-0.5)
        qn = nrm.tile([128, 512], F16, tag="qn", name="qn")
        nc.vector.tensor_scalar_mul(qn[:, 0:256], ps_qt[:, 0:256], rq[:, 0:1])
        nc.vector.tensor_scalar_mul(qn[:, 256:512], ps_qt[:, 256:512],
                                    rq[:, 1:2])
        state[("qn", i)] = qn

    def transp_q(i):
        qn = state.pop(("qn", i))
        ptkq = state.pop(("ptkq", i))
        for h in range(2):
            for c in range(2):
                nc.tensor.transpose(ptkq[:, 2 * c + h, :],
                                    qn[:, ts(2 * h + c, 128)], ident[:])
        if fast_scale:
            nc.vector.tensor_scalar_mul(QT[:, :, i, :, :], ptkq[:, 0:4, :],
                                        4.0)
        else:
            for h in range(2):
                for c in range(2):
                    nc.vector.tensor_scalar_mul(QT[:, c, i, h, :],
                                                ptkq[:, 2 * c + h, :],
                                                qs4_sb[:, ds(c, 1)])

    def att_scores_emit(i):
        """Scores + exp for tile i; both heads packed in N=256 per j-tile.
        Groups of 2 j-tiles (aligned to even j) per psum bank."""
        jlo = max(0, i - WT)
        pts = {}
        for jp in range(jlo // 2, i // 2 + 1):
            s_lo = max(jlo, 2 * jp) - 2 * jp
            s_hi = min(i, 2 * jp + 1) - 2 * jp
            stg = ps_att.tile([128, 2, 256], F32, tag="sa", name="stg")
            for s in range(s_lo, s_hi + 1):
                j = 2 * jp + s
                nc.tensor.matmul(stg[:, s], lhsT=KT[:, :, ts(j, 128)],
                                 rhs=QT[:, :, i, :, :],
                                 start=True, stop=False, perf_mode=DR)
                nc.tensor.matmul(stg[:, s], lhsT=KL[:, :, ts(j, 128)],
                                 rhs=QT[:, :, i, :, :],
                                 start=False, stop=True, perf_mode=DR)
            pt = ptp_pool.tile([128, 2, 256], F16, tag="pt", name="pt_exp")
            nsl = s_hi - s_lo + 1
            nc.scalar.activation(pt[:, ds(s_lo, nsl)], stg[:, ds(s_lo, nsl)],
                                 AF.Exp, bias=bias_m2[:], scale=1.0 / 256.0)
            pts[jp] = pt
        # sliding-window masks: zero the disallowed triangles post-exp (Pool)
        ptd = pts[i // 2][:, i % 2]       # diag tile: keep t_j <= t_i
        nc.gpsimd.affine_select(ptd, ptd, pattern=[[0, 2], [1, 128]],
                                compare_op=ALU.is_ge, fill=0.0,
                                channel_multiplier=-1)
        if i - WT >= 0:
            pte = pts[(i - WT) // 2][:, (i - WT) % 2]  # edge: keep t_j>=t_i+1
            nc.gpsimd.affine_select(pte, pte, pattern=[[0, 2], [-1, 128]],
                                    compare_op=ALU.is_ge, fill=0.0,
                                    base=-1, channel_multiplier=1)
        state[("pts", i)] = pts

    def att_pv_emit(i, h):
        jlo = max(0, i - WT)
        pts = state[("pts", i)]
        ps_o = ps_att.tile([128, 257], F32, tag="sa", name="ps_o")
        for j in range(jlo, i + 1):
            nc.tensor.matmul(ps_o[:], lhsT=pts[j // 2][:, j % 2,
                                                       ds(h * 128, 128)],
                             rhs=V[j][:], start=(j == jlo), stop=(j == i))
        if h == 1:
            state.pop(("pts", i))
        state[("po", i, h)] = ps_o

    def att_drain_emit(i, h):
        ps_o = state.pop(("po", i, h))
        if h == 0:
            a1u4 = aop.tile([128, 4, 128], F8, tag="a1", name="a1u4")
            s2u4 = aop.tile([128, 4, 128], F8, tag="s2", name="s2u4")
            a2 = aop.tile([128, 4, 128], F8, tag="a2", name="a2")
            state[("aop", i)] = (a1u4, s2u4, a2)
        else:
            a1u4, s2u4, a2 = state[("aop", i)]
        rr = stats.tile([128, 1], F32, tag="rr", name="rr")
        nc.vector.reciprocal(rr[:], ps_o[:, 256:257])
        ao = nrm.tile([128, 256], F16, tag="ao", name="ao")
        nc.vector.tensor_scalar_mul(ao[:], ps_o[:, 0:256], rr[:])
        ptao = ps_att.tile([128, 2, 128], F16, tag="sa", name="ptao")
        for c2 in range(2):
            nc.tensor.transpose(ptao[:, c2, :], ao[:, ts(c2, 128)], ident[:])
        hs = a1u4[:, ds(2 * h, 2), :]
        nc.vector.tensor_scalar_mul(hs, ptao[:], 4.0)
        ss = s2u4[:, ds(2 * h, 2), :]
        nc.vector.scalar_tensor_tensor(ss, ptao[:], 4.0, hs,
                                       ALU.mult, ALU.subtract)
        a2s = a2[:, ds(2 * h, 2), :]
        nc.gpsimd.tensor_scalar_mul(a2s, hs, 1.0 / 64.0)

    def op_region_a(j, nb):
        """First half (head-0 K rows) of an out-proj region chain."""
        a1u4, s2u4, a2 = state[("aop", j)]
        if nb == 0 and ("ob", j) not in state:
            ob = work.tile([128, 5, 512], F16, tag="ob", name="ob")
            state[("ob", j)] = ob
        ps3 = ps_out.tile([128, 512], F32, tag="o", name="ps3")
        state[("ps3", j, nb)] = ps3
        col = ds(nb * 512, 512)
        nc.tensor.matmul(ps3[:], lhsT=a1u4[:, 0:2, :],
                         rhs=wo4_sb[:, 0, :, col],
                         start=True, stop=False, perf_mode=DR)
        nc.tensor.matmul(ps3[:], lhsT=s2u4[:, 0:2, :],
                         rhs=wo4_sb[:, 0, :, col],
                         start=False, stop=False, perf_mode=DR)

    def op_region_b(j, nb):
        """Second half: head-1 rows, w2 correction, drain + DMA."""
        a1u4, s2u4, a2 = state[("aop", j)]
        ob = state[("ob", j)]
        ps3 = state.pop(("ps3", j, nb))
        col = ds(nb * 512, 512)
        nc.tensor.matmul(ps3[:], lhsT=a1u4[:, 2:4, :],
                         rhs=wo4_sb[:, 1, :, col],
                         start=False, stop=False, perf_mode=DR)
        nc.tensor.matmul(ps3[:], lhsT=s2u4[:, 2:4, :],
                         rhs=wo4_sb[:, 1, :, col],
                         start=False, stop=False, perf_mode=DR)
        for cp in range(2):
            nc.tensor.matmul(ps3[:], lhsT=a2[:, ds(2 * cp, 2), :],
                             rhs=w2_sb[:, cp, :, col],
                             start=False, stop=(cp == 1), perf_mode=DR)
        if j == nT - 1 and nb >= 3:
            # tail: split the drain across ACT||DVE to halve its latency
            nc.scalar.activation(ob[:, nb, 0:256], ps3[:, 0:256], AF.Copy,
                                 scale=1.0 / 16.0)
            nc.vector.tensor_scalar_mul(ob[:, nb, 256:512], ps3[:, 256:512],
                                        1.0 / 16.0)
        elif nb % 2 == 0:
            nc.scalar.activation(ob[:, nb, :], ps3[:], AF.Copy,
                                 scale=1.0 / 16.0)
        else:
            nc.vector.tensor_scalar_mul(ob[:, nb, :], ps3[:], 1.0 / 16.0)
        if j == nT - 1:
            # tail: per-region DMA so the last transfer starts ASAP
            nc.sync.dma_start(outp[ts(j, 128), ds(nb * 512, 512)],
                              ob[:, nb, :])
            if nb == 4:
                state.pop(("ob", j))
                state.pop(("aop", j))
        elif nb == 2:
            nc.sync.dma_start(outp[ts(j, 128), ds(0, 1536)], ob[:, 0:3, :])
        elif nb == 4:
            nc.sync.dma_start(outp[ts(j, 128), ds(1536, 1024)], ob[:, 3:5, :])
            state.pop(("ob", j))
            state.pop(("aop", j))

    def op_region(j, nb):
        op_region_a(j, nb)
        op_region_b(j, nb)

    def pop_jobs(k):
        for _ in range(k):
            if jobs:
                op_region(*jobs.popleft())

    def att_full(i):
        att_scores_emit(i)
        att_pv_emit(i, 0)
        att_pv_emit(i, 1)
        att_drain_emit(i, 0)
        att_drain_emit(i, 1)
        for nb in range(5):
            jobs.append((i, nb))

    # --- DMA priming: single SP queue, strict need order -------------------
    xt_i0 = work.tile([128, 2, nP, 2, 128], F8, tag="xt", name="xt_i")
    nc.sync.dma_start(xt_i0[:, 1], xcomb[0][:, 1])  # x0-hi
    xt_tiles[0] = xt_i0
    nc.sync.dma_start(wk8_sb[:], wk8)
    nc.sync.dma_start(wq8_sb[:, 0:5], wq8[:, 0:5])
    nc.sync.dma_start(wq8_sb[:, 5:nP], wq8[:, 5:nP])
    xt_i1 = work.tile([128, 2, nP, 2, 128], F8, tag="xt", name="xt_i")
    nc.sync.dma_start(xt_i1[:, 1], xcomb[1][:, 1])  # x1-hi
    xt_tiles[1] = xt_i1
    nc.sync.dma_start(wkc_sb[:], wkc)
    nc.sync.dma_start(xt_i0[:, 0], xcomb[0][:, 0])  # x0-lo
    nc.sync.dma_start(xt_i1[:, 0], xcomb[1][:, 0])  # x1-lo
    nc.sync.dma_start(wv8_sb[:], wv8)
    xt_dma_emit(2)
    nc.sync.dma_start(wvc_sb[:], wvc)
    nc.sync.dma_start(wqc_sb[:, 0:5], wqc[:, 0:5])
    nc.sync.dma_start(wqc_sb[:, 5:nP], wqc[:, 5:nP])
    xt_dma_emit(3)
    nc.sync.dma_start(wo4_sb[:], wo4)
    nc.sync.dma_start(w2_sb[:], w2)
    nc.sync.dma_start(qs_sb[:], qs)
    nc.sync.dma_start(ks_sb[:], ks)
    nc.vector.tensor_scalar_mul(qs4_sb[:], qs_sb[:], 4.0)
    nc.vector.tensor_scalar_mul(ks4_sb[:], ks_sb[:], 4.0)
    xt_dma_emit(4)

    # --- prologue: tiles 0-2, emission ordered by DMA arrival --------------
    kv_k_mains(0)
    q_mains(0)
    q_mains(1)
    kv_k_mains(1)
    kv_k_corrs(0)
    kv_k_corrs(1)
    kv_v_mains(0)
    kv_v_mains(1)
    kv_k_chain(2, pool=ps_out, tag="o")
    q_mains(2, pool=ps_out, tag="o")
    kv_v_mains(2)
    kv_v_corrs(0)
    kv_v_corrs(1)
    kv_v_corrs(2)
    norm_kv(0)
    transp_k(0)
    norm_kv(1)
    transp_k(1)
    norm_kv(2)
    q_corrs(0)
    norm_q(0)
    transp_q(0)
    transp_k(2)
    q_corrs(1)
    norm_q(1)
    transp_q(1)
    q_corrs(2)
    norm_q(2)
    transp_q(2)
    att_full(0)
    att_full(1)
    att_full(2)
    # kv chains run one tile ahead of the rest of the pipeline: tile i+1's
    # chains are emitted inside iteration i at its norm/exp stall points
    kv_k_chain(3)
    kv_v_chain(3)

    # --- steady loop -------------------------------------------------------
    NPOP = 8
    for i in range(3, nT):
        budget = 7 if len(jobs) > 8 else len(jobs)
        shares = [0] * NPOP
        for b in range(budget):
            shares[b % NPOP] += 1
        if i + 2 < nT:
            xt_dma_emit(i + 2)
        norm_kv(i)
        q_mains(i)          # PE filler while kn quant runs on DVE
        transp_k(i)
        pop_jobs(shares[2])
        q_corrs(i)
        pop_jobs(shares[3])
        norm_q(i)
        if i + 1 < nT:
            kv_k_chain(i + 1)   # PE filler while qn muls run on DVE
        pop_jobs(shares[5])
        transp_q(i)
        if i + 1 < nT:
            kv_v_chain(i + 1)   # PE filler while QT quant runs on DVE
        att_scores_emit(i)
        pop_jobs(shares[0])  # PE filler while exp runs on ACT
        pop_jobs(shares[6])
        att_pv_emit(i, 0)
        att_pv_emit(i, 1)    # hides ao(h0) DVE latency before transposes
        att_drain_emit(i, 0)
        att_drain_emit(i, 1)
        pop_jobs(shares[1])
        pop_jobs(shares[7])
        pop_jobs(shares[4])
        for nb in range(5):
            jobs.append((i, nb))
    # epilogue: tile 15's regions, head-0 halves overlapped with pv(h1)
    # (jobs list here holds exactly tile nT-1's five regions)
    jobs.clear()
    j = nT - 1
    op_region_a(j, 0)
    op_region_a(j, 1)
    op_region_b(j, 0)
    op_region_a(j, 2)
    op_region_b(j, 1)
    op_region_a(j, 3)
    op_region_b(j, 2)
    op_region_a(j, 4)
    op_region_b(j, 3)
    op_region_b(j, 4)


def _q8(x):
    import ml_dtypes
    return np.asarray(x, np.float32).astype(ml_dtypes.float8_e4m3)


def make_core_inputs(x, Wq, Wk, Wv, Wo, q_scale, k_scale, T=2048, D=2560):
    """Per-core input dicts (host-side sharding, quantization, layout)."""
    nT, nD = T // 128, D // 128
    nP = nD // 2
    qsv = np.ascontiguousarray(
        (1.0 + np.asarray(q_scale, np.float32)).reshape(2, 128).T)
    ksv = np.ascontiguousarray(
        (1.0 + np.asarray(k_scale, np.float32)).reshape(2, 128).T)

    def wsplit(W, s):
        """fp8 hi at scale s plus (W/16-partner, residual) correction pack.
        Returns (w8, wc) with layouts [128, nP, 2, N] / [128, nP, 2, 2, N]."""
        W = np.asarray(W, np.float32)
        Dd, N = W.shape
        w8 = _q8(s * W)
        w4 = _q8(w8.astype(np.float32) / 16.0)
        wlo = _q8(s * W - w8.astype(np.float32))
        w8r = np.ascontiguousarray(
            w8.reshape(nP, 2, 128, N).transpose(2, 0, 1, 3))
        wc = np.stack([w4, wlo])  # [2, D, N]
        wcr = np.ascontiguousarray(
            wc.reshape(2, nP, 2, 128, N).transpose(3, 1, 0, 2, 4))
        return w8r, wcr

    in_maps = []
    for core in range(8):
        b, G = core // 4, core % 4
        h0, h1 = G, G + 4
        Wq_s = np.concatenate(
            [Wq[:, 256 * h0:256 * (h0 + 1)], Wq[:, 256 * h1:256 * (h1 + 1)]], 1)
        Wk_s = np.asarray(Wk[:, 256 * G:256 * (G + 1)], np.float32)
        Wv_s = np.asarray(Wv[:, 256 * G:256 * (G + 1)], np.float32)
        Wo_s = np.concatenate(
            [Wo[256 * h0:256 * (h0 + 1)], Wo[256 * h1:256 * (h1 + 1)]], 0)

        wq8, wqc = wsplit(Wq_s, 64.0)
        wk8, wkc = wsplit(Wk_s, 64.0)
        wv8, wvc = wsplit(Wv_s, 256.0)

        wo4 = _q8(4.0 * Wo_s)
        w2 = _q8(256.0 * Wo_s - 64.0 * wo4.astype(np.float32))
        wo4r = np.ascontiguousarray(
            wo4.reshape(2, 2, 128, D).transpose(2, 0, 1, 3))
        w2r = np.ascontiguousarray(
            w2.reshape(2, 2, 128, D).transpose(2, 0, 1, 3))

        xT = np.asarray(x[b], np.float32).T  # [D, T]
        xh = _q8(xT)
        xlo = _q8(16.0 * (xT - xh.astype(np.float32)))
        xc = np.stack([xlo, xh])  # [2(w), D, T]
        xcomb = np.ascontiguousarray(
            xc.reshape(2, nP, 2, 128, nT, 128).transpose(4, 3, 0, 1, 2, 5))

        in_maps.append({
            "xcomb": xcomb,
            "wq8": wq8, "wqc": wqc, "wk8": wk8, "wkc": wkc,
            "wv8": wv8, "wvc": wvc, "wo4": wo4r, "w2": w2r,
            "qs": qsv, "ks": ksv,
        })
    return in_maps


_NC_CACHE = {}


def _get_nc(T=2048, D=2560, WIN=1024, fast_scale=True):
    key = (T, D, WIN, fast_scale)
    if key not in _NC_CACHE:
        _NC_CACHE[key] = build_nc(T, D, WIN, fast_scale)
    return _NC_CACHE[key]


def run_cores(inputs, trace=False):
    fast = (not np.any(np.asarray(inputs["q_scale"]))
            and not np.any(np.asarray(inputs["k_scale"])))
    nc = _get_nc(fast_scale=fast)
    in_maps = make_core_inputs(**inputs)
    res = run_bass_kernel_spmd(nc, in_maps, list(range(8)), trace=trace)
    B, T, D = inputs["x"].shape
    out = np.zeros((B, T, D), np.float32)
    for core in range(8):
        out[core // 4] += res.results[core]["outp"].astype(np.float32)
    return out, res


def kernel(x, Wq, Wk, Wv, Wo, q_scale, k_scale):
    out, _ = run_cores(dict(x=x, Wq=Wq, Wk=Wk, Wv=Wv, Wo=Wo,
                            q_scale=q_scale, k_scale=k_scale))
    return out


# revision 33
# speedup vs baseline: 1.0019x; 1.0019x over previous
"""Grouped-Query Attention (Gemma3-style, sliding-window) Trainium2 kernel.

Sharding: 8 cores = (batch b in {0,1}) x (kv-group G in {0..3}).
Each core computes, for its batch's tokens:
  - k/v projections for group G, q projections for heads {G, G+4},
  - qk-rmsnorm, sliding-window causal attention for its 2 heads,
  - partial output projection through the matching 512 rows of Wo.
Host sums the 4 partials per batch.

The q/k/v projections and the output projection run as fp8(e4m3)
DoubleRow matmuls (cost model: 0.5 cycles per output element, K=256 per
instruction) with full hi/lo residual correction:
  x = xh + xlo/16,  W = Whi/s + Wlo/s   (per-path scale s)
  x@W = xh@Whi + [xlo@(Whi/16) + xh@Wlo]   (lo*lo dropped)

Scores run fp8-DR with k-side residual correction only:
  q8 = q8(4*qhat), k8 = q8(4*khat), klo = q8(4*khat - k8)
  S = k8.T@q8 + klo.T@q8 = 16*qhat.khat + eps,  exp(S/256 - 2)
(quantized AFTER the PE transposes so no extra transpose work; one DR
per term per j-tile with both heads packed in N=256).  Numerics (CPU
emulation, matches HW to ~1%): this lands at max-rel-err 1.33e-2 vs
the 2e-2 gate; dropping any other correction pass (proj x/W, out-proj
a/W, fp8 PV) individually pushes past ~2.4e-2, so those stay.
Softmax/PV stay fp16.

Schedule/structure:
  - per-projection psum chains (k, v, q in separate 2-buf pools) +
    split rmsnorm so the startup only waits for the weights each chain
    actually needs; single SP DMA queue with need-ordered weight
    stream; tile-2's prologue chains borrow the idle out-proj psum
  - sliding-window masks applied as Pool affine_select zeroing on the
    exp output instead of PE mask-add matmuls
  - out-projection emitted as a job queue of [128,512] regions popped
    at the next iteration's known PE stall points (after norms/exp),
    so PSUM-drain and DVE-quant latency hide behind other PE work
  - pv(h1) runs before drain(h0) so the drain's DVE ao-scale latency
    hides under PE work; a2 quantization rides the idle Pool engine
  - output DMA: two [128,1536/1024] transfers per tile on the SP
    hardware DGE queue (per-region for the last tile to cut the tail)

Scales: proj psums hold 64*q, 64*k, 256*v (rmsnorm is scale-invariant,
V copy divides by 256). Out-proj psum holds 16*out; drain divides
by 16.
"""

import math
from collections import deque
from contextlib import ExitStack

import numpy as np

import concourse.bass as bass
import concourse.tile as tile
from concourse import bacc, mybir
from concourse.bass import ts, ds
from concourse.bass_utils import run_bass_kernel_spmd
from concourse.masks import make_identity

F8 = mybir.dt.float8e4
F16 = mybir.dt.float16
F32 = mybir.dt.float32
AF = mybir.ActivationFunctionType
ALU = mybir.AluOpType
DR = mybir.MatmulPerfMode.DoubleRow
_MY_FUNCS = {AF.Exp, AF.Ln, AF.Copy, AF.Square}

# Steer Bacc's activation-table chooser so Square/Ln/Exp/Copy all resolve to
# the one function set that contains them all (natural_log_exp_and_others).
import concourse.bacc as _bacc_mod
from concourse.hw_specs import get_activation_tables as _orig_gat

_ONE_SET = "natural_log_exp_and_others"


def _steered_gat(arch):
    tabs = _orig_gat(arch)
    if _ONE_SET not in tabs:
        return tabs
    return {name: (set(funcs) if name == _ONE_SET else set(funcs) - _MY_FUNCS)
            for name, funcs in tabs.items()}


_bacc_mod.get_activation_tables = _steered_gat

EPS = 1e-6
HD = 256  # head dim
EXP_BIAS = -2.0


def build_nc(T=2048, D=2560, WIN=1024, fast_scale=True):
    nT, nD, WT = T // 128, D // 128, WIN // 128
    nP = nD // 2  # chunk pairs
    nc = bacc.Bacc("TRN2", target_bir_lowering=False, debug=False)

    xcomb = nc.dram_tensor("xcomb", [nT, 128, 2, nP, 2, 128], F8,
                           kind="ExternalInput").ap()
    wq8 = nc.dram_tensor("wq8", [128, nP, 2, 512], F8, kind="ExternalInput").ap()
    wqc = nc.dram_tensor("wqc", [128, nP, 2, 2, 512], F8, kind="ExternalInput").ap()
    wk8 = nc.dram_tensor("wk8", [128, nP, 2, 256], F8, kind="ExternalInput").ap()
    wkc = nc.dram_tensor("wkc", [128, nP, 2, 2, 256], F8, kind="ExternalInput").ap()
    wv8 = nc.dram_tensor("wv8", [128, nP, 2, 256], F8, kind="ExternalInput").ap()
    wvc = nc.dram_tensor("wvc", [128, nP, 2, 2, 256], F8, kind="ExternalInput").ap()
    wo4 = nc.dram_tensor("wo4", [128, 2, 2, D], F8, kind="ExternalInput").ap()
    w2 = nc.dram_tensor("w2", [128, 2, 2, D], F8, kind="ExternalInput").ap()
    qs = nc.dram_tensor("qs", [128, 2], F32, kind="ExternalInput").ap()
    ks = nc.dram_tensor("ks", [128, 2], F32, kind="ExternalInput").ap()
    outp = nc.dram_tensor("outp", [T, D], F16, kind="ExternalOutput").ap()

    with tile.TileContext(nc) as tc, ExitStack() as ctx:
        _body(ctx, tc, nT, nP, WT, D, fast_scale,
              xcomb, wq8, wqc, wk8, wkc, wv8, wvc, wo4, w2, qs, ks, outp)

    nc.compile()
    return nc


def _body(ctx, tc, nT, nP, WT, D, fast_scale,
          xcomb, wq8, wqc, wk8, wkc, wv8, wvc, wo4, w2, qs, ks, outp):
    nc = tc.nc

    const = ctx.enter_context(tc.tile_pool(name="const", bufs=1))
    acts = ctx.enter_context(tc.tile_pool(name="acts", bufs=1))
    work = ctx.enter_context(tc.tile_pool(name="work", bufs=4))
    nrm = ctx.enter_context(tc.tile_pool(name="nrm", bufs=2))
    ptp_pool = ctx.enter_context(tc.tile_pool(name="ptp", bufs=8))
    aop = ctx.enter_context(tc.tile_pool(name="aop", bufs=3))
    stats = ctx.enter_context(tc.tile_pool(name="stats", bufs=8))
    # 8 PSUM banks: kv 2 + q 2 + att (stg/po/ptao/ptkq rotation) 2 + out 2.
    # Tile-2's prologue chains borrow the (still unused) out slots so the
    # startup can run 3 tiles of projection lookahead with 2-buf pools.
    ps_kv = ctx.enter_context(tc.tile_pool(name="ps_kv", bufs=2,
                                           space="PSUM"))
    ps_q = ctx.enter_context(tc.tile_pool(name="ps_q", bufs=2, space="PSUM"))
    ps_att = ctx.enter_context(tc.tile_pool(name="ps_att", bufs=2,
                                            space="PSUM"))
    ps_out = ctx.enter_context(tc.tile_pool(name="ps_out", bufs=2,
                                            space="PSUM"))

    ident = const.tile([128, 128], F16, tag="ident")
    make_identity(nc, ident[:])
    bias_eps = const.tile([128, 1], F32, tag="bias_eps")
    nc.vector.memset(bias_eps[:], EPS)
    bias_m2 = const.tile([128, 1], F32, tag="bias_m2")
    nc.vector.memset(bias_m2[:], EXP_BIAS)
    qs_sb = const.tile([128, 2], F32, tag="qs")
    ks_sb = const.tile([128, 2], F32, tag="ks")
    qs4_sb = const.tile([128, 2], F32, tag="qs4")
    ks4_sb = const.tile([128, 2], F32, tag="ks4")

    # one tile per weight tensor: few big DMAs beat many chunked ones
    wq8_sb = const.tile([128, nP, 2, 512], F8, tag="wq8")
    wk8_sb = const.tile([128, nP, 2, 256], F8, tag="wk8")
    wv8_sb = const.tile([128, nP, 2, 256], F8, tag="wv8")
    wqc_sb = const.tile([128, nP, 2, 2, 512], F8, tag="wqc")
    wkc_sb = const.tile([128, nP, 2, 2, 256], F8, tag="wkc")
    wvc_sb = const.tile([128, nP, 2, 2, 256], F8, tag="wvc")
    wq8_c = [wq8_sb[:, c] for c in range(nP)]
    wk8_c = [wk8_sb[:, c] for c in range(nP)]
    wv8_c = [wv8_sb[:, c] for c in range(nP)]
    wqc_c = [wqc_sb[:, c] for c in range(nP)]
    wkc_c = [wkc_sb[:, c] for c in range(nP)]
    wvc_c = [wvc_sb[:, c] for c in range(nP)]
    wo4_sb = const.tile([128, 2, 2, D], F8, tag="wo4")
    w2_sb = const.tile([128, 2, 2, D], F8, tag="w2")

    # full-length fp8 q/k for the scores (hi at scale 4, k residual lo);
    # numerics: scores = q8@k8 + q8@klo, exp scale 1/256 (rel-err 1.35e-2)
    # QT layout: [d, c(2), i, h(2), t]  (c = head-dim chunk)
    QT = acts.tile([128, 2, nT, 2, 128], F8, tag="QT")
    KT = acts.tile([128, 2, nT * 128], F8, tag="KT")   # [d, c, t] hi
    KL = acts.tile([128, 2, nT * 128], F8, tag="KL")   # [d, c, t] residual
    V = [acts.tile([128, 257], F16, tag=f"v{j}", name=f"v{j}")
         for j in range(nT)]  # last column is ones (softmax row sums)
    for j in range(nT):
        nc.gpsimd.memset(V[j][:, 256:257], 1.0)

    state = {}
    xt_tiles = {}
    jobs = deque()

    def xt_dma_emit(i):
        xt_i = work.tile([128, 2, nP, 2, 128], F8, tag="xt", name="xt_i")
        # hi half first: main-pass matmuls only need xh
        nc.sync.dma_start(xt_i[:, 1], xcomb[i][:, 1])
        nc.sync.dma_start(xt_i[:, 0], xcomb[i][:, 0])
        xt_tiles[i] = xt_i

    def kv_k_mains(i, pool=None, tag="pkv"):
        xt_i = xt_tiles[i]
        ps_kvt = (pool or ps_kv).tile([128, 512], F32, tag=tag, name="ps_kvt")
        sl = ps_kvt[:, 0:256]
        for cp in range(nP):
            nc.tensor.matmul(sl, lhsT=xt_i[:, 1, cp, :, :],
                             rhs=wk8_c[cp], start=(cp == 0), stop=False,
                             perf_mode=DR)
        state[("pskv", i)] = ps_kvt

    def kv_k_corrs(i):
        xt_i = xt_tiles[i]
        sl = state[("pskv", i)][:, 0:256]
        for c in range(2 * nP):
            cp, p = c // 2, c % 2
            nc.tensor.matmul(sl, lhsT=xt_i[:, :, cp, p, :],
                             rhs=wkc_c[cp][:, :, p, :], start=False,
                             stop=(c == 2 * nP - 1), perf_mode=DR)

    def kv_k_chain(i, pool=None, tag="pkv"):
        kv_k_mains(i, pool, tag)
        kv_k_corrs(i)

    def kv_v_mains(i):
        xt_i = xt_tiles[i]
        sl = state[("pskv", i)][:, 256:512]
        for cp in range(nP):
            nc.tensor.matmul(sl, lhsT=xt_i[:, 1, cp, :, :],
                             rhs=wv8_c[cp], start=(cp == 0), stop=False,
                             perf_mode=DR)

    def kv_v_corrs(i):
        xt_i = xt_tiles[i]
        sl = state[("pskv", i)][:, 256:512]
        for c in range(2 * nP):
            cp, p = c // 2, c % 2
            nc.tensor.matmul(sl, lhsT=xt_i[:, :, cp, p, :],
                             rhs=wvc_c[cp][:, :, p, :], start=False,
                             stop=(c == 2 * nP - 1), perf_mode=DR)

    def kv_v_chain(i):
        kv_v_mains(i)
        kv_v_corrs(i)

    def q_mains(i, pool=None, tag="pq2"):
        xt_i = xt_tiles[i]
        ps_qt = (pool or ps_q).tile([128, 512], F32, tag=tag, name="ps_qt")
        for cp in range(nP):
            nc.tensor.matmul(ps_qt[:], lhsT=xt_i[:, 1, cp, :, :],
                             rhs=wq8_c[cp], start=(cp == 0), stop=False,
                             perf_mode=DR)
        state[("psq", i)] = ps_qt

    def q_corrs(i):
        xt_i = xt_tiles.pop(i)
        ps_qt = state[("psq", i)]
        for c in range(2 * nP):
            cp, p = c // 2, c % 2
            nc.tensor.matmul(ps_qt[:], lhsT=xt_i[:, :, cp, p, :],
                             rhs=wqc_c[cp][:, :, p, :], start=False,
                             stop=(c == 2 * nP - 1), perf_mode=DR)

    def norm_kv(i):
        ps_kvt = state.pop(("pskv", i))
        # V tile: psum holds 256*v
        nc.vector.tensor_scalar_mul(V[i][:, 0:256], ps_kvt[:, 256:512],
                                    1.0 / 256.0)
        # k rmsnorm (scale-invariant): rinv = exp(-0.5*ln(ssq/256 + eps))
        sstk = stats.tile([128, 1], F32, tag="sstk", name="sstk")
        sq = nrm.tile([128, 256], F32, tag="sq", name="sq")
        nc.scalar.activation(sq[:], ps_kvt[:, 0:256], AF.Square,
                             accum_out=sstk[:])
        lnk = stats.tile([128, 1], F32, tag="lnk", name="lnk")
        nc.scalar.activation(lnk[:], sstk[:], AF.Ln, bias=bias_eps[:],
                             scale=1.0 / 256.0)
        rk = stats.tile([128, 1], F32, tag="rk", name="rk")
        nc.scalar.activation(rk[:], lnk[:], AF.Exp, scale=-0.5)
        kn = nrm.tile([128, 256], F16, tag="kn", name="kn")
        nc.vector.tensor_scalar_mul(kn[:], ps_kvt[:, 0:256], rk[:])
        state[("kn", i)] = kn

    def transp_k(i):
        kn = state.pop(("kn", i))
        ptkq = ps_att.tile([128, 6, 128], F16, tag="sa", name="ptkq")
        for cc in range(2):
            nc.tensor.transpose(ptkq[:, 4 + cc, :], kn[:, ts(cc, 128)],
                                ident[:])
        # quantize after transpose: k8 = q8(4*k), klo = q8(4*k - k8)
        if fast_scale:
            nc.vector.tensor_scalar_mul(KT[:, :, ts(i, 128)],
                                        ptkq[:, 4:6, :], 4.0)
            nc.vector.scalar_tensor_tensor(KL[:, :, ts(i, 128)],
                                           ptkq[:, 4:6, :], 4.0,
                                           KT[:, :, ts(i, 128)],
                                           ALU.mult, ALU.subtract)
        else:
            for cc in range(2):
                nc.vector.tensor_scalar_mul(KT[:, cc, ts(i, 128)],
                                            ptkq[:, 4 + cc, :],
                                            ks4_sb[:, ds(cc, 1)])
                nc.vector.scalar_tensor_tensor(KL[:, cc, ts(i, 128)],
                                               ptkq[:, 4 + cc, :],
                                               ks4_sb[:, ds(cc, 1)],
                                               KT[:, cc, ts(i, 128)],
                                               ALU.mult, ALU.subtract)
        state[("ptkq", i)] = ptkq

    def norm_q(i):
        ps_qt = state.pop(("psq", i))
        sstq = stats.tile([128, 2], F32, tag="sstq", name="sstq")
        for jj in range(2):
            sq = nrm.tile([128, 256], F32, tag="sq", name="sq")
            nc.scalar.activation(sq[:], ps_qt[:, ts(jj, 256)], AF.Square,
                                 accum_out=sstq[:, jj:jj + 1])
        lnq = stats.tile([128, 2], F32, tag="lnq", name="lnq")
        nc.scalar.activation(lnq[:], sstq[:], AF.Ln, bias=bias_eps[:],
                             scale=1.0 / 256.0)
        rq = stats.tile([128, 2], F32, tag="rq", name="rq")
        nc.scalar.activation(rq[:], lnq[:], AF.Exp, scale=-0.5)
        qn = nrm.tile([128, 512], F16, tag="qn", name="qn")
        nc.vector.tensor_scalar_mul(qn[:, 0:256], ps_qt[:, 0:256], rq[:, 0:1])
        nc.vector.tensor_scalar_mul(qn[:, 256:512], ps_qt[:, 256:512],
                                    rq[:, 1:2])
        state[("qn", i)] = qn

    def transp_q(i):
        qn = state.pop(("qn", i))
        ptkq = state.pop(("ptkq", i))
        for h in range(2):
            for c in range(2):
                nc.tensor.transpose(ptkq[:, 2 * c + h, :],
                                    qn[:, ts(2 * h + c, 128)], ident[:])
        if fast_scale:
            nc.vector.tensor_scalar_mul(QT[:, :, i, :, :], ptkq[:, 0:4, :],
                                        4.0)
        else:
            for h in range(2):
                for c in range(2):
                    nc.vector.tensor_scalar_mul(QT[:, c, i, h, :],
                                                ptkq[:, 2 * c + h, :],
                                                qs4_sb[:, ds(c, 1)])

    def att_scores_emit(i):
        """Scores + exp for tile i; both heads packed in N=256 per j-tile.
        Groups of 2 j-tiles (aligned to even j) per psum bank."""
        jlo = max(0, i - WT)
        pts = {}
        for jp in range(jlo // 2, i // 2 + 1):
            s_lo = max(jlo, 2 * jp) - 2 * jp
            s_hi = min(i, 2 * jp + 1) - 2 * jp
            stg = ps_att.tile([128, 2, 256], F32, tag="sa", name="stg")
            for s in range(s_lo, s_hi + 1):
                j = 2 * jp + s
                nc.tensor.matmul(stg[:, s], lhsT=KT[:, :, ts(j, 128)],
                                 rhs=QT[:, :, i, :, :],
                                 start=True, stop=False, perf_mode=DR)
                nc.tensor.matmul(stg[:, s], lhsT=KL[:, :, ts(j, 128)],
                                 rhs=QT[:, :, i, :, :],
                                 start=False, stop=True, perf_mode=DR)
            pt = ptp_pool.tile([128, 2, 256], F16, tag="pt", name="pt_exp")
            nsl = s_hi - s_lo + 1
            nc.scalar.activation(pt[:, ds(s_lo, nsl)], stg[:, ds(s_lo, nsl)],
                                 AF.Exp, bias=bias_m2[:], scale=1.0 / 256.0)
            pts[jp] = pt
        # sliding-window masks: zero the disallowed triangles post-exp (Pool)
        ptd = pts[i // 2][:, i % 2]       # diag tile: keep t_j <= t_i
        nc.gpsimd.affine_select(ptd, ptd, pattern=[[0, 2], [1, 128]],
                                compare_op=ALU.is_ge, fill=0.0,
                                channel_multiplier=-1)
        if i - WT >= 0:
            pte = pts[(i - WT) // 2][:, (i - WT) % 2]  # edge: keep t_j>=t_i+1
            nc.gpsimd.affine_select(pte, pte, pattern=[[0, 2], [-1, 128]],
                                    compare_op=ALU.is_ge, fill=0.0,
                                    base=-1, channel_multiplier=1)
        state[("pts", i)] = pts

    def att_pv_emit(i, h):
        jlo = max(0, i - WT)
        pts = state[("pts", i)]
        ps_o = ps_att.tile([128, 257], F32, tag="sa", name="ps_o")
        for j in range(jlo, i + 1):
            nc.tensor.matmul(ps_o[:], lhsT=pts[j // 2][:, j % 2,
                                                       ds(h * 128, 128)],
                             rhs=V[j][:], start=(j == jlo), stop=(j == i))
        if h == 1:
            state.pop(("pts", i))
        state[("po", i, h)] = ps_o

    def att_drain_emit(i, h):
        ps_o = state.pop(("po", i, h))
        if h == 0:
            a1u4 = aop.tile([128, 4, 128], F8, tag="a1", name="a1u4")
            s2u4 = aop.tile([128, 4, 128], F8, tag="s2", name="s2u4")
            a2 = aop.tile([128, 4, 128], F8, tag="a2", name="a2")
            state[("aop", i)] = (a1u4, s2u4, a2)
        else:
            a1u4, s2u4, a2 = state[("aop", i)]
        rr = stats.tile([128, 1], F32, tag="rr", name="rr")
        nc.vector.reciprocal(rr[:], ps_o[:, 256:257])
        ao = nrm.tile([128, 256], F16, tag="ao", name="ao")
        nc.vector.tensor_scalar_mul(ao[:], ps_o[:, 0:256], rr[:])
        ptao = ps_att.tile([128, 2, 128], F16, tag="sa", name="ptao")
        for c2 in range(2):
            nc.tensor.transpose(ptao[:, c2, :], ao[:, ts(c2, 128)], ident[:])
        hs = a1u4[:, ds(2 * h, 2), :]
        nc.vector.tensor_scalar_mul(hs, ptao[:], 4.0)
        ss = s2u4[:, ds(2 * h, 2), :]
        nc.vector.scalar_tensor_tensor(ss, ptao[:], 4.0, hs,
                                       ALU.mult, ALU.subtract)
        a2s = a2[:, ds(2 * h, 2), :]
        nc.gpsimd.tensor_scalar_mul(a2s, hs, 1.0 / 64.0)

    def op_region_a(j, nb):
        """First half (head-0 K rows) of an out-proj region chain."""
        a1u4, s2u4, a2 = state[("aop", j)]
        if nb == 0 and ("ob", j) not in state:
            ob = work.tile([128, 5, 512], F16, tag="ob", name="ob")
            state[("ob", j)] = ob
        ps3 = ps_out.tile([128, 512], F32, tag="o", name="ps3")
        state[("ps3", j, nb)] = ps3
        col = ds(nb * 512, 512)
        nc.tensor.matmul(ps3[:], lhsT=a1u4[:, 0:2, :],
                         rhs=wo4_sb[:, 0, :, col],
                         start=True, stop=False, perf_mode=DR)
        nc.tensor.matmul(ps3[:], lhsT=s2u4[:, 0:2, :],
                         rhs=wo4_sb[:, 0, :, col],
                         start=False, stop=False, perf_mode=DR)

    def op_region_b(j, nb):
        """Second half: head-1 rows, w2 correction, drain + DMA."""
        a1u4, s2u4, a2 = state[("aop", j)]
        ob = state[("ob", j)]
        ps3 = state.pop(("ps3", j, nb))
        col = ds(nb * 512, 512)
        nc.tensor.matmul(ps3[:], lhsT=a1u4[:, 2:4, :],
                         rhs=wo4_sb[:, 1, :, col],
                         start=False, stop=False, perf_mode=DR)
        nc.tensor.matmul(ps3[:], lhsT=s2u4[:, 2:4, :],
                         rhs=wo4_sb[:, 1, :, col],
                         start=False, stop=False, perf_mode=DR)
        for cp in range(2):
            nc.tensor.matmul(ps3[:], lhsT=a2[:, ds(2 * cp, 2), :],
                             rhs=w2_sb[:, cp, :, col],
                             start=False, stop=(cp == 1), perf_mode=DR)
        if nb % 2 == 0:
            nc.scalar.activation(ob[:, nb, :], ps3[:], AF.Copy,
                                 scale=1.0 / 16.0)
        else:
            nc.vector.tensor_scalar_mul(ob[:, nb, :], ps3[:], 1.0 / 16.0)
        if j == nT - 1:
            # tail: per-region DMA so the last transfer starts ASAP
            nc.sync.dma_start(outp[ts(j, 128), ds(nb * 512, 512)],
                              ob[:, nb, :])
            if nb == 4:
                state.pop(("ob", j))
                state.pop(("aop", j))
        elif nb == 2:
            nc.sync.dma_start(outp[ts(j, 128), ds(0, 1536)], ob[:, 0:3, :])
        elif nb == 4:
            nc.sync.dma_start(outp[ts(j, 128), ds(1536, 1024)], ob[:, 3:5, :])
            state.pop(("ob", j))
            state.pop(("aop", j))

    def op_region(j, nb):
        op_region_a(j, nb)
        op_region_b(j, nb)

    def pop_jobs(k):
        for _ in range(k):
            if jobs:
                op_region(*jobs.popleft())

    def att_full(i):
        att_scores_emit(i)
        att_pv_emit(i, 0)
        att_pv_emit(i, 1)
        att_drain_emit(i, 0)
        att_drain_emit(i, 1)
        for nb in range(5):
            jobs.append((i, nb))

    # --- DMA priming: single SP queue, strict need order -------------------
    xt_i0 = work.tile([128, 2, nP, 2, 128], F8, tag="xt", name="xt_i")
    nc.sync.dma_start(xt_i0[:, 1], xcomb[0][:, 1])  # x0-hi
    xt_tiles[0] = xt_i0
    nc.sync.dma_start(wk8_sb[:], wk8)
    nc.sync.dma_start(wq8_sb[:, 0:5], wq8[:, 0:5])
    nc.sync.dma_start(wq8_sb[:, 5:nP], wq8[:, 5:nP])
    xt_i1 = work.tile([128, 2, nP, 2, 128], F8, tag="xt", name="xt_i")
    nc.sync.dma_start(xt_i1[:, 1], xcomb[1][:, 1])  # x1-hi
    xt_tiles[1] = xt_i1
    nc.sync.dma_start(wkc_sb[:], wkc)
    nc.sync.dma_start(xt_i0[:, 0], xcomb[0][:, 0])  # x0-lo
    nc.sync.dma_start(xt_i1[:, 0], xcomb[1][:, 0])  # x1-lo
    nc.sync.dma_start(wv8_sb[:], wv8)
    xt_dma_emit(2)
    nc.sync.dma_start(wvc_sb[:], wvc)
    nc.sync.dma_start(wqc_sb[:, 0:5], wqc[:, 0:5])
    nc.sync.dma_start(wqc_sb[:, 5:nP], wqc[:, 5:nP])
    xt_dma_emit(3)
    nc.sync.dma_start(wo4_sb[:], wo4)
    nc.sync.dma_start(w2_sb[:], w2)
    nc.sync.dma_start(qs_sb[:], qs)
    nc.sync.dma_start(ks_sb[:], ks)
    nc.vector.tensor_scalar_mul(qs4_sb[:], qs_sb[:], 4.0)
    nc.vector.tensor_scalar_mul(ks4_sb[:], ks_sb[:], 4.0)
    xt_dma_emit(4)

    # --- prologue: tiles 0-2, emission ordered by DMA arrival --------------
    kv_k_mains(0)
    q_mains(0)
    q_mains(1)
    kv_k_mains(1)
    kv_k_corrs(0)
    kv_k_corrs(1)
    kv_v_mains(0)
    kv_v_mains(1)
    kv_k_chain(2, pool=ps_out, tag="o")
    q_mains(2, pool=ps_out, tag="o")
    kv_v_mains(2)
    kv_v_corrs(0)
    kv_v_corrs(1)
    kv_v_corrs(2)
    norm_kv(0)
    transp_k(0)
    norm_kv(1)
    transp_k(1)
    norm_kv(2)
    q_corrs(0)
    norm_q(0)
    transp_q(0)
    transp_k(2)
    q_corrs(1)
    norm_q(1)
    transp_q(1)
    q_corrs(2)
    norm_q(2)
    transp_q(2)
    att_full(0)
    att_full(1)
    att_full(2)
    # kv chains run one tile ahead of the rest of the pipeline: tile i+1's
    # chains are emitted inside iteration i at its norm/exp stall points
    kv_k_chain(3)
    kv_v_chain(3)

    # --- steady loop -------------------------------------------------------
    NPOP = 8
    for i in range(3, nT):
        budget = 7 if len(jobs) > 8 else len(jobs)
        shares = [0] * NPOP
        for b in range(budget):
            shares[b % NPOP] += 1
        if i + 2 < nT:
            xt_dma_emit(i + 2)
        norm_kv(i)
        q_mains(i)          # PE filler while kn quant runs on DVE
        transp_k(i)
        pop_jobs(shares[2])
        q_corrs(i)
        pop_jobs(shares[3])
        norm_q(i)
        if i + 1 < nT:
            kv_k_chain(i + 1)   # PE filler while qn muls run on DVE
        pop_jobs(shares[5])
        transp_q(i)
        if i + 1 < nT:
            kv_v_chain(i + 1)   # PE filler while QT quant runs on DVE
        att_scores_emit(i)
        pop_jobs(shares[0])  # PE filler while exp runs on ACT
        pop_jobs(shares[6])
        att_pv_emit(i, 0)
        att_pv_emit(i, 1)    # hides ao(h0) DVE latency before transposes
        att_drain_emit(i, 0)
        att_drain_emit(i, 1)
        pop_jobs(shares[1])
        pop_jobs(shares[7])
        pop_jobs(shares[4])
        for nb in range(5):
            jobs.append((i, nb))
    # epilogue: tile 15's regions, head-0 halves overlapped with pv(h1)
    # (jobs list here holds exactly tile nT-1's five regions)
    jobs.clear()
    j = nT - 1
    op_region_a(j, 0)
    op_region_a(j, 1)
    op_region_b(j, 0)
    op_region_a(j, 2)
    op_region_b(j, 1)
    op_region_a(j, 3)
    op_region_b(j, 2)
    op_region_a(j, 4)
    op_region_b(j, 3)
    op_region_b(j, 4)


def _q8(x):
    import ml_dtypes
    return np.asarray(x, np.float32).astype(ml_dtypes.float8_e4m3)


def make_core_inputs(x, Wq, Wk, Wv, Wo, q_scale, k_scale, T=2048, D=2560):
    """Per-core input dicts (host-side sharding, quantization, layout)."""
    nT, nD = T // 128, D // 128
    nP = nD // 2
    qsv = np.ascontiguousarray(
        (1.0 + np.asarray(q_scale, np.float32)).reshape(2, 128).T)
    ksv = np.ascontiguousarray(
        (1.0 + np.asarray(k_scale, np.float32)).reshape(2, 128).T)

    def wsplit(W, s):
        """fp8 hi at scale s plus (W/16-partner, residual) correction pack.
        Returns (w8, wc) with layouts [128, nP, 2, N] / [128, nP, 2, 2, N]."""
        W = np.asarray(W, np.float32)
        Dd, N = W.shape
        w8 = _q8(s * W)
        w4 = _q8(w8.astype(np.float32) / 16.0)
        wlo = _q8(s * W - w8.astype(np.float32))
        w8r = np.ascontiguousarray(
            w8.reshape(nP, 2, 128, N).transpose(2, 0, 1, 3))
        wc = np.stack([w4, wlo])  # [2, D, N]
        wcr = np.ascontiguousarray(
            wc.reshape(2, nP, 2, 128, N).transpose(3, 1, 0, 2, 4))
        return w8r, wcr

    in_maps = []
    for core in range(8):
        b, G = core // 4, core % 4
        h0, h1 = G, G + 4
        Wq_s = np.concatenate(
            [Wq[:, 256 * h0:256 * (h0 + 1)], Wq[:, 256 * h1:256 * (h1 + 1)]], 1)
        Wk_s = np.asarray(Wk[:, 256 * G:256 * (G + 1)], np.float32)
        Wv_s = np.asarray(Wv[:, 256 * G:256 * (G + 1)], np.float32)
        Wo_s = np.concatenate(
            [Wo[256 * h0:256 * (h0 + 1)], Wo[256 * h1:256 * (h1 + 1)]], 0)

        wq8, wqc = wsplit(Wq_s, 64.0)
        wk8, wkc = wsplit(Wk_s, 64.0)
        wv8, wvc = wsplit(Wv_s, 256.0)

        wo4 = _q8(4.0 * Wo_s)
        w2 = _q8(256.0 * Wo_s - 64.0 * wo4.astype(np.float32))
        wo4r = np.ascontiguousarray(
            wo4.reshape(2, 2, 128, D).transpose(2, 0, 1, 3))
        w2r = np.ascontiguousarray(
            w2.reshape(2, 2, 128, D).transpose(2, 0, 1, 3))

        xT = np.asarray(x[b], np.float32).T  # [D, T]
        xh = _q8(xT)
        xlo = _q8(16.0 * (xT - xh.astype(np.float32)))
        xc = np.stack([xlo, xh])  # [2(w), D, T]
        xcomb = np.ascontiguousarray(
            xc.reshape(2, nP, 2, 128, nT, 128).transpose(4, 3, 0, 1, 2, 5))

        in_maps.append({
            "xcomb": xcomb,
            "wq8": wq8, "wqc": wqc, "wk8": wk8, "wkc": wkc,
            "wv8": wv8, "wvc": wvc, "wo4": wo4r, "w2": w2r,
            "qs": qsv, "ks": ksv,
        })
    return in_maps


_NC_CACHE = {}


def _get_nc(T=2048, D=2560, WIN=1024, fast_scale=True):
    key = (T, D, WIN, fast_scale)
    if key not in _NC_CACHE:
        _NC_CACHE[key] = build_nc(T, D, WIN, fast_scale)
    return _NC_CACHE[key]


def run_cores(inputs, trace=False):
    fast = (not np.any(np.asarray(inputs["q_scale"]))
            and not np.any(np.asarray(inputs["k_scale"])))
    nc = _get_nc(fast_scale=fast)
    in_maps = make_core_inputs(**inputs)
    res = run_bass_kernel_spmd(nc, in_maps, list(range(8)), trace=trace)
    B, T, D = inputs["x"].shape
    out = np.zeros((B, T, D), np.float32)
    for core in range(8):
        out[core // 4] += res.results[core]["outp"].astype(np.float32)
    return out, res


def kernel(x, Wq, Wk, Wv, Wo, q_scale, k_scale):
    out, _ = run_cores(dict(x=x, Wq=Wq, Wk=Wk, Wv=Wv, Wo=Wo,
                            q_scale=q_scale, k_scale=k_scale))
    return out
